# revision 19
# baseline (speedup 1.0000x reference)
"""MDGRec GNN message-passing kernel for 8 Trainium2 NeuronCores.

Strategy (SPMD, one NEFF on 8 cores):
  - Nodes row-sharded: core m owns dst rows [m*18750, (m+1)*18750).
  - Host relabels nodes with a permutation pi so that each core's bin-packed
    128-row groups occupy contiguous rows of a padded 19200-row shard; all
    device-side writes/reads become contiguous slice DMAs.
  - id and text features concatenated into 128-wide rows (256B bf16 rows).
  - Layer tables (full [153600, 128] in pi-space) built via on-device
    AllGather between layers.
  - SpMM per layer: bulk dma_gather of h[edge_col] (int16 indices, pi-space
    split into 5 ranges of 30720) spread round-robin over all 4 SWDGE queues,
    one-hot segment matrices with edge values folded in are built on the
    HOST (bf16) and streamed from DRAM on the Scalar-engine HWDGE ring (so
    small gather-gating loads on the SP ring never queue behind them),
    segment-sum via PE matmuls in PSUM.
  - Fused epilogue (layer mean, tail amp, gate, blend) on device, with
    PSUM-side copies/scales on the Scalar engine and h0+h1 accumulation done
    by identity matmuls on the PE (Vector engine kept nearly idle).

The edge template (identical instruction stream across cores): per core,
G groups x 5 ranges x C_GR chunks of 128 edges, supergroups of S_G groups
share one dma_gather call per range.
"""

import os
import numpy as np
import ml_dtypes

import concourse.bass as bass
import concourse.bacc as bacc
import concourse.tile as tile
import concourse.mybir as mybir
from concourse import bass_utils, library_config
from concourse.masks import make_identity

# ---- problem constants (hardcoded per spec) ----
N_NODES = 150000
EMB_DIM = 64
TEXT_DIM = 384
NCORES = 8
SHARD = N_NODES // NCORES          # 18750 real rows per core
F = 2 * EMB_DIM                    # 128 concat feature width

# ---- template constants ----
G = 150                            # groups per core
S_G = 3                            # groups per supergroup
N_SG = G // S_G                    # 50
SHARD_P = G * 128                  # 19200 padded rows per core (pi-space)
TBL_ROWS = NCORES * SHARD_P        # 153600 pi-space nodes
N_RANGE = 5
RANGE_SIZE = TBL_ROWS // N_RANGE   # 30720 (int16-safe)
C_GR = 7                           # chunks per (group, range)
CPG = N_RANGE * C_GR               # 35 chunks per group
C_SG = S_G * CPG                   # 105 chunks per supergroup
CALL_CH = S_G * C_GR               # 21 chunks per gather call
CALL_IDX = CALL_CH * 128           # 2688 idxs per gather call
CAP_R = C_GR * 128                 # 896 edge capacity per (group, range)
B_P = 6                            # groups per prologue iteration
VQ = 1.0                           # edge-value quantization scale (bf16: none)

_CACHE = {}
_LAST_IN_MAPS = None


# ======================================================================
# device program
# ======================================================================

def _build(n_sg_run=N_SG, run_layers=(0, 1), do_collectives=True,
           single_core=False):
    fp32 = mybir.dt.float32
    bf16 = mybir.dt.bfloat16
    i16 = mybir.dt.int16
    u8 = mybir.dt.uint8

    if single_core:
        do_collectives = False
    nc = bacc.Bacc("TRN2", target_bir_lowering=False, debug=False,
                   num_devices=1 if single_core else NCORES,
                   num_swdge_queues=4)

    # inputs (per core)
    text_T = nc.dram_tensor("text_T", [TEXT_DIM, SHARD_P], bf16, kind="ExternalInput")
    id_shard = nc.dram_tensor("id_shard", [SHARD_P, EMB_DIM], fp32, kind="ExternalInput")
    gidx = nc.dram_tensor("gidx", [N_SG, 128, N_RANGE * (CALL_IDX // 16)], i16,
                          kind="ExternalInput")
    s_mats = nc.dram_tensor("s_mats", [N_SG, 128, C_SG * 128], bf16,
                            kind="ExternalInput")
    aux_a = nc.dram_tensor("aux_a", [N_SG, 128, S_G], fp32, kind="ExternalInput")
    w_text = nc.dram_tensor("w_text", [TEXT_DIM, EMB_DIM], fp32, kind="ExternalInput")
    b_text = nc.dram_tensor("b_text", [1, EMB_DIM], fp32, kind="ExternalInput")
    w_fuse = nc.dram_tensor("w_fuse", [F, EMB_DIM], fp32, kind="ExternalInput")
    b_fuse = nc.dram_tensor("b_fuse", [EMB_DIM, 1], fp32, kind="ExternalInput")

    out = nc.dram_tensor("out", [SHARD_P, EMB_DIM], fp32, kind="ExternalOutput")

    # internal DRAM
    cat_bf = nc.dram_tensor("cat_bf", [SHARD_P, F], bf16)
    h1_bf = nc.dram_tensor("h1_bf", [SHARD_P, F], bf16)
    table0 = nc.dram_tensor("table0", [TBL_ROWS, F], bf16, addr_space="Shared")
    table1 = nc.dram_tensor("table1", [TBL_ROWS, F], bf16, addr_space="Shared")

    with tile.TileContext(nc) as tc:
        nc.gpsimd.load_library(library_config.mlp)
        with (
            tc.tile_pool(name="const", bufs=1) as cpool,
            tc.tile_pool(name="sb", bufs=3) as sb,
            tc.tile_pool(name="sp", bufs=3) as sp,
            tc.tile_pool(name="xp", bufs=3) as xp,
            tc.tile_pool(name="psum", bufs=2, space="PSUM") as ps,
            tc.tile_pool(name="psep", bufs=2, space="PSUM") as pse,
        ):
            # ---- constants ----
            ident = cpool.tile([128, 128], fp32, tag="ident")
            make_identity(nc, ident[:])
            ident255_bf = cpool.tile([128, 128], bf16, tag="identbf")
            nc.scalar.activation(ident255_bf[:], ident[:],
                                 mybir.ActivationFunctionType.Copy, scale=VQ)
            wt_f = cpool.tile([128, 3 * EMB_DIM], fp32, tag="wtf")
            for k in range(3):
                nc.sync.dma_start(wt_f[:, k * EMB_DIM:(k + 1) * EMB_DIM],
                                  w_text[k * 128:(k + 1) * 128, :])
            wt_t = cpool.tile([128, 3 * EMB_DIM], bf16, tag="wt")
            nc.vector.tensor_copy(wt_t[:], wt_f[:])
            ones_t = cpool.tile([1, 128], bf16, tag="ones")
            nc.vector.memset(ones_t[:], 1.0)
            btf_t = cpool.tile([1, EMB_DIM], fp32, tag="btf")
            nc.sync.dma_start(btf_t[:], b_text[:])
            bt_t = cpool.tile([1, EMB_DIM], bf16, tag="bt")
            nc.vector.tensor_copy(bt_t[:], btf_t[:])
            wf_t = cpool.tile([128, EMB_DIM], fp32, tag="wf")
            nc.sync.dma_start(wf_t[:], w_fuse[:])
            bf_t = cpool.tile([EMB_DIM, 1], fp32, tag="bf")
            nc.sync.dma_start(bf_t[:], b_fuse[:])
            nidx_reg = nc.gpsimd.to_reg(CALL_IDX)

            # global pi-space is [half][core][rows-in-half] so that each
            # half-AllGather writes a contiguous table slice
            HROWS = SHARD_P // 2
            GROWS = NCORES * HROWS

            # ---- text projection + cat_bf assembly (pi-layout), batched ----
            for it in range(G // B_P):
                r0 = it * B_P * 128
                tx3 = sb.tile([128, 3, B_P * 128], bf16, tag="tx3")
                nc.sync.dma_start(
                    tx3[:],
                    text_T[:, r0:r0 + B_P * 128].rearrange("(k p) c -> p k c", k=3))
                id_t = sb.tile([128, B_P, EMB_DIM], fp32, tag="idt")
                nc.sync.dma_start(
                    id_t[:],
                    id_shard[r0:r0 + B_P * 128, :].rearrange("(g p) e -> p g e", g=B_P))
                catb = sb.tile([128, B_P, F], bf16, tag="catb")
                for g in range(B_P):
                    proj_ps = ps.tile([128, EMB_DIM], fp32, tag="mm")
                    for k in range(3):
                        nc.tensor.matmul(proj_ps[:],
                                         lhsT=tx3[:, k, g * 128:(g + 1) * 128],
                                         rhs=wt_t[:, k * EMB_DIM:(k + 1) * EMB_DIM],
                                         start=(k == 0), stop=False)
                    nc.tensor.matmul(proj_ps[:], lhsT=ones_t[:], rhs=bt_t[:],
                                     start=False, stop=True)
                    nc.scalar.activation(catb[:, g, 0:EMB_DIM], id_t[:, g, :],
                                         mybir.ActivationFunctionType.Copy)
                    nc.scalar.activation(catb[:, g, EMB_DIM:F], proj_ps[:],
                                         mybir.ActivationFunctionType.Copy)
                nc.sync.dma_start(
                    cat_bf[r0:r0 + B_P * 128, :].rearrange("(g p) e -> p g e", g=B_P),
                    catb[:])
                # first half done: AllGather it while projecting the rest
                if do_collectives and (it + 1) * B_P * 128 >= HROWS and it * B_P * 128 < HROWS:
                    nc.gpsimd.collective_compute(
                        "AllGather", mybir.AluOpType.bypass,
                        replica_groups=[list(range(NCORES))],
                        ins=[cat_bf[0:HROWS, :]],
                        outs=[table0[0:GROWS, :]],
                    )

            # ---- AllGather h0 (second half) ----
            if do_collectives:
                nc.gpsimd.collective_compute(
                    "AllGather", mybir.AluOpType.bypass,
                    replica_groups=[list(range(NCORES))],
                    ins=[cat_bf[HROWS:, :]],
                    outs=[table0[GROWS:, :]],
                )

            # ---- SpMM layers ----
            for layer in run_layers:
                table = table0 if layer == 0 else table1
                for sg in range(n_sg_run):
                    aux_t = sb.tile([128, S_G], fp32, tag="aux")
                    nc.sync.dma_start(aux_t[:], aux_a[sg, :, :])
                    gi = sb.tile([128, N_RANGE * (CALL_IDX // 16)], i16, tag="gi")
                    nc.sync.dma_start(gi[:], gidx[sg, :, :])
                    S_t = sp.tile([128, C_SG, 128], bf16, tag="S")
                    # scalar (ACT) HWDGE ring: keeps these bulk loads out of
                    # the SP FIFO so gi/aux never queue behind them
                    for s3 in range(S_G):
                        nc.scalar.dma_start(
                            S_t[:, s3 * CPG:(s3 + 1) * CPG, :],
                            s_mats[sg, :, s3 * CPG * 128:(s3 + 1) * CPG * 128])

                    Xsr = []
                    W16 = CALL_IDX // 16
                    for r in range(N_RANGE):
                        X = xp.tile([128, CALL_CH, F], bf16, tag=f"X{r}")
                        nc.gpsimd.dma_gather(
                            X[:],
                            table[r * RANGE_SIZE:(r + 1) * RANGE_SIZE, :],
                            gi[:, r * W16:(r + 1) * W16], CALL_IDX, nidx_reg, F,
                            single_packet=False,
                            queue_num=(sg * N_RANGE + r) % 4)
                        Xsr.append(X)

                    for s in range(S_G):
                        g = sg * S_G + s
                        r0 = g * 128
                        acc = ps.tile([128, F], fp32, tag="mm")
                        chunks = [(r, s * C_GR + c)
                                  for r in range(N_RANGE) for c in range(C_GR)]
                        for j, (r, k) in enumerate(chunks):
                            c = k - s * C_GR
                            ci = s * CPG + r * C_GR + c
                            nc.tensor.matmul(acc[:], lhsT=S_t[:, ci, :],
                                             rhs=Xsr[r][:, k, :],
                                             start=(j == 0),
                                             stop=(j == CPG - 1) and layer == 0)

                        if layer == 0:
                            resb = sb.tile([128, F], bf16, tag="resb")
                            nc.scalar.activation(resb[:], acc[:],
                                                 mybir.ActivationFunctionType.Copy,
                                                 scale=1.0 / VQ)
                            nc.sync.dma_start(h1_bf[r0:r0 + 128, :], resb[:])
                        else:
                            # fused epilogue for this group's rows
                            # acc currently holds VQ * (A @ h1); add VQ*h0 and
                            # VQ*h1 via 255*identity matmuls so the whole acc
                            # is VQ * (h0 + h1 + h2).
                            h0_t = sb.tile([128, F], bf16, tag="h0")
                            nc.sync.dma_start(h0_t[:], cat_bf[r0:r0 + 128, :])
                            h1_t = sb.tile([128, F], bf16, tag="h1")
                            nc.sync.dma_start(h1_t[:], h1_bf[r0:r0 + 128, :])
                            nc.tensor.matmul(acc[:], lhsT=ident255_bf[:],
                                             rhs=h0_t[:], start=False, stop=False)
                            nc.tensor.matmul(acc[:], lhsT=ident255_bf[:],
                                             rhs=h1_t[:], start=False, stop=True)

                            # fsum halves with layer-mean / tail-amp scales
                            # (aux already folds the 1/VQ)
                            fsum = sb.tile([128, F], fp32, tag="fsum")
                            nc.scalar.activation(fsum[:, 0:EMB_DIM],
                                                 acc[:, 0:EMB_DIM],
                                                 mybir.ActivationFunctionType.Copy,
                                                 scale=1.0 / (3.0 * VQ))
                            nc.scalar.activation(fsum[:, EMB_DIM:F],
                                                 acc[:, EMB_DIM:F],
                                                 mybir.ActivationFunctionType.Copy,
                                                 scale=aux_t[:, s:s + 1])

                            tp = pse.tile([128, 128], fp32, tag="tp")
                            nc.tensor.transpose(out=tp[:], in_=fsum[:],
                                                identity=ident[:])
                            ft = sb.tile([128, 128], fp32, tag="ft")
                            nc.scalar.activation(ft[:], tp[:],
                                                 mybir.ActivationFunctionType.Copy)

                            gp = pse.tile([EMB_DIM, 128], fp32, tag="gp")
                            nc.tensor.matmul(gp[:], lhsT=wf_t[:], rhs=ft[:],
                                             start=True, stop=True)
                            gate_T = sb.tile([EMB_DIM, 128], fp32, tag="gateT")
                            nc.scalar.activation(gate_T[:], gp[:],
                                                 mybir.ActivationFunctionType.Sigmoid,
                                                 bias=bf_t[:, :1])
                            g2 = pse.tile([128, EMB_DIM], fp32, tag="g2")
                            nc.tensor.transpose(out=g2[:], in_=gate_T[:],
                                                identity=ident[0:EMB_DIM, 0:EMB_DIM])
                            gate = sb.tile([128, EMB_DIM], fp32, tag="gate")
                            nc.scalar.activation(gate[:], g2[:],
                                                 mybir.ActivationFunctionType.Copy)

                            dif = sb.tile([128, EMB_DIM], fp32, tag="dif")
                            nc.vector.tensor_tensor(out=dif[:],
                                                    in0=fsum[:, 0:EMB_DIM],
                                                    in1=fsum[:, EMB_DIM:F],
                                                    op=mybir.AluOpType.subtract)
                            nc.vector.tensor_tensor(out=dif[:], in0=dif[:],
                                                    in1=gate[:],
                                                    op=mybir.AluOpType.mult)
                            fused = sb.tile([128, EMB_DIM], fp32, tag="fused")
                            nc.vector.tensor_tensor(out=fused[:],
                                                    in0=fsum[:, EMB_DIM:F],
                                                    in1=dif[:],
                                                    op=mybir.AluOpType.add)
                            nc.sync.dma_start(out[r0:r0 + 128, :], fused[:])

                    # h1 first half complete: AllGather it under remaining L0
                    if (layer == 0 and do_collectives and 1 in run_layers
                            and (sg + 1) * S_G * 128 >= HROWS
                            and sg * S_G * 128 < HROWS):
                        nc.gpsimd.collective_compute(
                            "AllGather", mybir.AluOpType.bypass,
                            replica_groups=[list(range(NCORES))],
                            ins=[h1_bf[0:HROWS, :]],
                            outs=[table1[0:GROWS, :]],
                        )

                if layer == 0 and do_collectives and 1 in run_layers:
                    nc.gpsimd.collective_compute(
                        "AllGather", mybir.AluOpType.bypass,
                        replica_groups=[list(range(NCORES))],
                        ins=[h1_bf[HROWS:, :]],
                        outs=[table1[GROWS:, :]],
                    )

    nc.compile()
    return nc


# ======================================================================
# host preprocessing
# ======================================================================

def _preprocess(edge_row, edge_col, edge_val, tail_mask, amp):
    """Full host-side preprocessing. Two passes:
    1. per-core packing of dst rows into groups by total degree (snake on
       sorted degrees), defining the pi permutation; then exact per-
       (group, src-range) bucket counts are checked against CAP_R and
       repaired by moving rows between groups (ranges live in pi-space, so
       they are only known once pi exists — repair breaks the cycle).
    2. per-core edge template fill (gather indices, one-hot segment mats)."""
    # ---- pass 1: pack by total degree, then repair ----
    grp = np.empty(N_NODES, np.int64)
    slot = np.empty(N_NODES, np.int64)
    deg_t = np.bincount(edge_row, minlength=N_NODES)
    for m in range(NCORES):
        lo = m * SHARD
        dt_ = deg_t[lo:lo + SHARD]
        order = np.argsort(-dt_, kind="stable")
        # snake assignment balances totals; 125 rows per group
        gassign = np.empty(SHARD, np.int64)
        idx = np.arange(SHARD)
        rounds = idx // G
        posr = idx % G
        fwd = (rounds % 2 == 0)
        gassign[order] = np.where(fwd, posr, G - 1 - posr)
        grp[lo:lo + SHARD] = gassign
        # slots within group by row id order
        o2 = np.lexsort((np.arange(SHARD), gassign))
        sg_sorted = gassign[o2]
        starts = np.searchsorted(sg_sorted, np.arange(G))
        sl = np.arange(SHARD) - starts[sg_sorted]
        slot_l = np.empty(SHARD, np.int64)
        slot_l[o2] = sl
        slot[lo:lo + SHARD] = slot_l

    def _pi_global(core, p_local):
        """pi-space layout [half][core][rows] so half-AllGathers write
        contiguous table slices; p_local = grp*128 + slot in [0, SHARD_P)."""
        hrows = SHARD_P // 2
        h = p_local // hrows
        return h * (NCORES * hrows) + core * hrows + (p_local - h * hrows)

    pi = _pi_global(np.arange(N_NODES) // SHARD, grp * 128 + slot)

    # ---- check/repair (group, range) capacities per core ----
    pc = pi[edge_col]
    rng_id = pc // RANGE_SIZE
    for m in range(NCORES):
        lo = m * SHARD
        sel = (edge_row >= lo) & (edge_row < lo + SHARD)
        er = edge_row[sel] - lo
        rr = rng_id[sel]
        gg = grp[lo + er]
        for _ in range(50):
            cnt = np.zeros((G, N_RANGE), np.int64)
            np.add.at(cnt, (gg, rr), 1)
            over = np.argwhere(cnt > CAP_R)
            if len(over) == 0:
                break
            nrows = np.bincount(grp[lo:lo + SHARD], minlength=G)
            # move one row out of each overflowing (g, r) to a group w/ room
            deg_gr = np.zeros((SHARD, N_RANGE), np.int64)
            np.add.at(deg_gr, (er, rr), 1)
            for g_o, r_o in over:
                rows_g = np.where(grp[lo:lo + SHARD] == g_o)[0]
                rows_g = rows_g[np.argsort(-deg_gr[rows_g, r_o])]
                moved = False
                need = cnt[g_o, r_o] - CAP_R
                for row in rows_g:
                    if deg_gr[row, r_o] == 0:
                        break
                    for g_n in np.argsort(cnt[:, r_o]):
                        if g_n == g_o or nrows[g_n] >= 128:
                            continue
                        if np.all(cnt[g_n] + deg_gr[row] <= CAP_R):
                            cnt[g_o] -= deg_gr[row]
                            cnt[g_n] += deg_gr[row]
                            nrows[g_o] -= 1
                            nrows[g_n] += 1
                            grp[lo + row] = g_n
                            gg = grp[lo + er]
                            moved = True
                            break
                    need = cnt[g_o, r_o] - CAP_R
                    if need <= 0:
                        break
                if not moved and cnt[g_o, r_o] > CAP_R:
                    raise RuntimeError("capacity repair failed")
            # recompute slots for this core after moves
            gassign = grp[lo:lo + SHARD]
            o2 = np.lexsort((np.arange(SHARD), gassign))
            sg_sorted = gassign[o2]
            starts = np.searchsorted(sg_sorted, np.arange(G))
            sl = np.arange(SHARD) - starts[sg_sorted]
            slot_l = np.empty(SHARD, np.int64)
            slot_l[o2] = sl
            slot[lo:lo + SHARD] = slot_l
        else:
            raise RuntimeError("repair loop did not converge")
        pi = _pi_global(np.arange(N_NODES) // SHARD, grp * 128 + slot)
        pc = pi[edge_col]
        rng_id = pc // RANGE_SIZE

    # ---- pass 2: per-core template fill ----
    cores = []
    for m in range(NCORES):
        lo = m * SHARD
        sel = (edge_row >= lo) & (edge_row < lo + SHARD)
        er = edge_row[sel] - lo
        ev = edge_val[sel].astype(np.float32)
        e_pc = pc[sel]                      # pi-space col
        e_r = (e_pc // RANGE_SIZE).astype(np.int64)
        e_cloc = (e_pc - e_r * RANGE_SIZE).astype(np.int64)
        e_g = grp[lo + er]
        e_slot = slot[lo + er]

        bucket = e_g * N_RANGE + e_r
        eorder = np.argsort(bucket, kind="stable")
        b_sorted = bucket[eorder]
        cnt = np.bincount(b_sorted, minlength=G * N_RANGE)
        assert cnt.max() <= CAP_R, cnt.max()
        off = np.zeros(G * N_RANGE + 1, np.int64)
        np.cumsum(cnt, out=off[1:])
        pos = np.arange(len(eorder)) - off[b_sorted]

        so_g = e_g[eorder]
        so_r = e_r[eorder]
        e_sg = so_g // S_G
        e_s = so_g % S_G
        e_c = pos // 128
        e_p = pos % 128
        e_ci = e_s * CPG + so_r * C_GR + e_c

        # host-built one-hot segment matrices (uint8-quantized edge values):
        # s_mats[sg, p, ci*128 + slot] = round(val*VQ)  (pads stay zero)
        s_arr = np.zeros((N_SG, 128, C_SG * 128), ml_dtypes.bfloat16)
        lin_s = ((e_sg * 128 + e_p) * C_SG + e_ci) * 128 + e_slot[eorder]
        s_arr.reshape(-1)[lin_s] = ev[eorder].astype(ml_dtypes.bfloat16)

        gidx16 = np.zeros((N_SG, N_RANGE, 16, CALL_IDX // 16), np.int16)
        e_k = e_s * C_GR + e_c
        q = e_k * 128 + e_p
        lin2 = ((e_sg * N_RANGE + so_r) * 16 + (q % 16)) * (CALL_IDX // 16) + (q // 16)
        gidx16.reshape(-1)[lin2] = e_cloc[eorder].astype(np.int16)
        gidx_arr = np.tile(gidx16, (1, 1, 8, 1))          # [N_SG, 5, 128, W16]
        gidx_arr = np.ascontiguousarray(
            gidx_arr.transpose(0, 2, 1, 3).reshape(N_SG, 128, -1))

        pi_l = grp[lo:lo + SHARD] * 128 + slot[lo:lo + SHARD]  # local padded pos
        tf_p = np.full(SHARD_P, 1.0 / (3.0 * VQ), np.float32)
        tmask = tail_mask[lo:lo + SHARD].astype(bool)
        tf_p[pi_l] = np.where(tmask, amp, 1.0).astype(np.float32) / (3.0 * VQ)
        # tailf columns: aux[sg, p, s] = tf for row (sg*S_G+s)*128+p
        aux_arr = np.ascontiguousarray(
            tf_p.reshape(G, 128).reshape(N_SG, S_G, 128).transpose(0, 2, 1))

        cores.append({
            "gidx": gidx_arr, "aux_a": aux_arr, "pi_l": pi_l, "s_mats": s_arr,
        })
    return cores


def kernel(text_feats, edge_row, edge_col, edge_val, tail_mask, user_emb,
           item_emb, W_text, b_text, W_fuse, b_fuse, tail_amp):
    text_feats = np.asarray(text_feats, np.float32)
    edge_row = np.asarray(edge_row).astype(np.int64)
    edge_col = np.asarray(edge_col).astype(np.int64)
    edge_val = np.asarray(edge_val, np.float32)
    tail_mask = np.asarray(tail_mask).astype(bool)
    user_emb = np.asarray(user_emb, np.float32)
    item_emb = np.asarray(item_emb, np.float32)
    W_text = np.asarray(W_text, np.float32)
    b_text = np.asarray(b_text, np.float32)
    W_fuse = np.asarray(W_fuse, np.float32)
    b_fuse = np.asarray(b_fuse, np.float32)
    amp = float(1.0 + 1.0 / (1.0 + np.exp(-np.float64(np.asarray(tail_amp)))))

    emb_id = np.concatenate([user_emb, item_emb], axis=0)  # [N, 64]

    if "nc" not in _CACHE:
        _CACHE["nc"] = _build()
    nc = _CACHE["nc"]

    b_text_row = b_text[None, :].astype(np.float32)
    b_fuse_col = b_fuse[:, None].astype(np.float32)

    cores = _preprocess(edge_row, edge_col, edge_val, tail_mask, amp)

    in_maps = []
    for m in range(NCORES):
        pre = cores[m]
        lo = m * SHARD
        pi_l = pre["pi_l"]
        text_p = np.zeros((SHARD_P, TEXT_DIM), ml_dtypes.bfloat16)
        text_p[pi_l] = text_feats[lo:lo + SHARD].astype(ml_dtypes.bfloat16)
        id_p = np.zeros((SHARD_P, EMB_DIM), np.float32)
        id_p[pi_l] = emb_id[lo:lo + SHARD]
        in_maps.append({
            "text_T": np.ascontiguousarray(text_p.T),
            "id_shard": id_p,
            "gidx": pre["gidx"], "aux_a": pre["aux_a"],
            "s_mats": pre["s_mats"],
            "w_text": W_text, "b_text": b_text_row,
            "w_fuse": W_fuse, "b_fuse": b_fuse_col,
        })

    global _LAST_IN_MAPS
    _LAST_IN_MAPS = in_maps
    res = bass_utils.run_bass_kernel_spmd(nc, in_maps, core_ids=list(range(NCORES)))

    out = np.empty((N_NODES, EMB_DIM), np.float32)
    for m in range(NCORES):
        lo = m * SHARD
        out[lo:lo + SHARD] = res.results[m]["out"][cores[m]["pi_l"]]
    return out
